# revision 23
# baseline (speedup 1.0000x reference)
"""Bass/Trainium2 kernel for nn_BQAVariant (BQA: basis-weighted KV attention).

Reference computation (B=2, T=2048, D=768, H=12 q-heads, KH=4 KV basis
heads, HD=64):
  q = x@wq; k_basis = x@wk; v_basis = x@wv
  w = softmax(alpha); k/v = einsum('hj,btjd->bthd', w, {k,v}_basis)
  q,k = rmsnorm(rope(q,k)) * 1.2
  y = causal_sdpa(q, k, v, scale=HD**-0.5) @ wo

Sharding: 24 (batch, head) pairs over 8 cores -> core c handles batch c//4
and heads {3g, 3g+1, 3g+2} with g = c%4.  The alpha-softmax basis
combination is folded on the host into effective per-head wk/wv, so each
core runs three independent standard attention heads.  Each core emits its
partial c_proj output (transposed, [768, 2048]); the host sums the 4
partials per batch.

v2 design (bf16 data, fp32 PSUM accumulation), ~158us/iter vs 176us for
the fp32r baseline under identical in-process difference timing:
  A) QKV projections from pre-transposed bf16 xT with packed [768, 576]
     bf16 weights; PSUM evicted once to bf16; RMS sum-of-squares computed
     PRE-rope (rotation preserves norms; cos^2+sin^2=1); fused q+k RoPE
     (3 DVE ops); rsqrt computed ENTIRELY on DVE (reciprocal + 2
     Babylonian sqrt iterations) so ScalarE runs nothing but Exp/Copy ->
     exactly ONE activation-table load for the whole kernel (Sqrt lives in
     a different ACT table set than Exp; in the interleaved schedule each
     sqrt<->exp alternation cost a ~2.7us table reload -- the fp32r
     baseline paid 11 loads, a naive interleave 33).  Q^T/K^T written
     DUPLICATED across both partition halves (normalize-mul writes each
     head twice; one PE transpose per (q/k, head) with a contiguous
     128-wide stationary).
  B) Transposed flash attention: consecutive causal k-tiles (j, j+1) run
     CONCURRENTLY in disjoint PE row-groups (tile_position via base
     partition 0/64 of the duplicated Q^T/K^T halves) -> ~2x S matmul
     throughput.  exp() on ScalarE in [128,2,512] two-bank batches; V'
     carries a ones column so the softmax denominator falls out of the PV
     matmul; the denominator reciprocal is partition-broadcast with a
     K=1 ones matmul.
  C) c_proj: three K=64 matmuls accumulating one PSUM bank; eviction
     alternates ScalarE/VectorE.  Stage A emission is interleaved with
     attention chunks (4 t-tiles, then one 512-wide q-chunk) so
     PE/DVE/ACT/GpSimd overlap across phases.  8 PSUM banks:
     proj 1 + projv 1 + tr/pb 1 + scores 2x2 + po/pp 1.
"""

import os
import sys

sys.path.insert(0, "/opt/trn_rl_repo")

import numpy as np
import ml_dtypes

import concourse.bass as bass
import concourse.tile as tile
from concourse import bacc, mybir

# Steer Bacc.insert_act_table_loads toward the one table set that serves
# BOTH Exp and Ln (natural_log_exp_and_others): hide Exp/Ln from every
# other set so the load-insertion fixpoint cannot pick exp_and_others /
# natural_log and thrash ~2.7us table reloads on each Ln<->Exp
# alternation.  The emitted act_func_set_id still indexes the real
# act_info.json list, so the NEFF loads a genuine table that contains
# both functions.
_orig_gat = bacc.get_activation_tables


def _gat_combined(arch):
    out = {}
    for name, funcs in _orig_gat(arch).items():
        f = set(funcs)
        if name != "natural_log_exp_and_others":
            f.discard(mybir.ActivationFunctionType.Exp)
            f.discard(mybir.ActivationFunctionType.Ln)
        out[name] = f
    return out


bacc.get_activation_tables = _gat_combined
from concourse.bass import ts
from concourse.bass_utils import run_bass_kernel_spmd
from concourse.masks import make_identity

F32 = mybir.dt.float32
F32R = mybir.dt.float32r
BF16 = mybir.dt.bfloat16
NPBF16 = ml_dtypes.bfloat16

B, T, D = 2, 2048, 768
H, KH, HD = 12, 4, 64
HPC = 3            # heads per core
NCORES = 8
EPS = 1e-6
QK = 1.2
NTT = T // 128     # 16 T-tiles
NKD = D // 128     # 6 contraction tiles for projections
QCH = 512          # max q-chunk width in flash stage (PSUM bank limit)
# uneven q-chunk widths: big chunks early (where stage A overlaps), small
# final chunk so the un-overlappable exp tail shrinks; all multiples of 128
_CW = [int(x) for x in
       os.environ.get("BQA_CHUNKS", "512,512,512,512").split(",")]
assert sum(_CW) == T and all(w % 128 == 0 and w <= QCH for w in _CW)
CHUNKS = []
_q0 = 0
for _w in _CW:
    CHUNKS.append((_q0, _w))
    _q0 += _w
WQW = 576          # packed projection width: 192 q | 192 k | 192 v

# analysis knob: "all" | "a" (projections only) | "b" (attention+cproj only)
STAGES = os.environ.get("BQA_STAGES", "all")
# hardware-timing knob: repeat the whole body N times inside a For_i loop
REPEAT = int(os.environ.get("BQA_REPEAT", "1"))
# dup-transpose strategy: "bcast" = one PE transpose with stride-0 doubled
# stationary; "two" = two transposes (lower + upper psum half)
DUPT = os.environ.get("BQA_DUPT", "bcast")
# engine for the rmsnorm multiply: "vector" | "gpsimd"
NORMED_ENG = os.environ.get("BQA_NORMED", "vector")
# sum-of-squares: "ttr" (fused tensor_tensor_reduce) | "sq" (mul + reduce)
SUMSQ = os.environ.get("BQA_SUMSQ", "sq")
# cproj contraction: "3stream" (three K=64 matmuls) | "stacked" (K=128+64,
# head 1 moved to upper partitions by SBUF->SBUF DMA)
CPROJ = os.environ.get("BQA_CPROJ", "3stream")
# rstd path: "lnexp" (2 ScalarE ops via exp(-0.5*ln(x)); Ln+Exp share the
# natural_log_exp_and_others table set with the attention Exp, so no table
# thrash) | "dve" (reciprocal + 2 Babylonian sqrt iterations, ~12 DVE ops)
RSTD = os.environ.get("BQA_RSTD", "dve")
# engine for the rope multiplies/add: "vector" | "gpsimd"
ROPE_ENG = os.environ.get("BQA_ROPE", "vector")
# engines for stage-A PSUM evictions: comma list of which go to ScalarE
# (subset of "qk,vp,qkt"); the rest stay on DVE
EVICT_ACT = set(filter(None, os.environ.get("BQA_EVICT_ACT", "").split(",")))
# stage-A group size: 0 = legacy per-tile path; 1/2/4 = batch the sumsq /
# reduce / rstd-chain ops over groups of tiles (identical arithmetic, far
# fewer small DVE instructions)
GRP = int(os.environ.get("BQA_GRP", "0"))
# exp slicing from the pair's first diagonal offset (1) vs full pair (0)
C0MIN = int(os.environ.get("BQA_C0MIN", "0"))
# cproj eviction: 1 = phase-aware (ACT early chunks, DVE late), 0 = m%2
PHEV = int(os.environ.get("BQA_PHEV", "0"))


def build_nc(repeat=None, stages=None):
    repeat = REPEAT if repeat is None else repeat
    stages = STAGES if stages is None else stages
    nc = bacc.Bacc(None, target_bir_lowering=False)

    xT = nc.declare_dram_parameter("xT", [D, T], BF16, isOutput=False)
    wqkv = nc.declare_dram_parameter("wqkv", [D, WQW], BF16, isOutput=False)
    wo01 = nc.declare_dram_parameter("wo01", [128, D], BF16, isOutput=False)
    wo2 = nc.declare_dram_parameter("wo2", [64, D], BF16, isOutput=False)
    wo3 = nc.declare_dram_parameter("wo3", [64, HPC * D], BF16, isOutput=False)
    csn = nc.declare_dram_parameter("csn", [T, HD], BF16, isOutput=False)
    scn = nc.declare_dram_parameter("scn", [T, HD], BF16, isOutput=False)
    masks = nc.declare_dram_parameter("masks", [128, 128], BF16, isOutput=False)
    outT = nc.declare_dram_parameter("outT", [D, T], F32, isOutput=True)

    with tile.TileContext(nc) as tc:
        with (
            tc.tile_pool(name="persist", bufs=1) as persist,
            tc.tile_pool(name="misc", bufs=2) as misc,
            tc.tile_pool(name="p_sb", bufs=3) as p_pool,
            tc.tile_pool(name="co_sb", bufs=2) as co_pool,
            # PSUM: 8 banks: psqk 1 + [psv|tr] 1 + ps_s 2x2 + [po|pb|pp] 2x1
            tc.tile_pool(name="proj_ps", bufs=1, space="PSUM") as proj_ps,
            tc.tile_pool(name="tr_ps", bufs=1, space="PSUM") as tr_ps,
            tc.tile_pool(name="attn_ps", bufs=2, space="PSUM") as attn_ps,
            tc.tile_pool(name="opp_ps", bufs=2, space="PSUM") as opp_ps,
        ):
            # --- persistent SBUF tensors ---
            wqkv_sb = persist.tile([128, NKD, WQW], BF16)
            wqkv_r = wqkv.rearrange("(k p) n -> p k n", p=128)
            for k in range(NKD):
                eng = nc.sync if k % 2 == 0 else nc.gpsimd
                eng.dma_start(out=wqkv_sb[:, k, :], in_=wqkv_r[:, k, :])

            if CPROJ == "stacked":
                wo01_sb = persist.tile([128, D], BF16)
                wo2_sb = persist.tile([64, D], BF16)
                nc.sync.dma_start(out=wo01_sb, in_=wo01[:])
                nc.sync.dma_start(out=wo2_sb, in_=wo2[:])
            else:
                wo_d = persist.tile([64, HPC, D], BF16)
                nc.sync.dma_start(
                    out=wo_d, in_=wo3.rearrange("p (h d) -> p h d", h=HPC))

            csn_sb = persist.tile([128, NTT, HD], BF16)
            scn_sb = persist.tile([128, NTT, HD], BF16)
            nc.sync.dma_start(out=csn_sb, in_=csn.rearrange("(i p) d -> p i d", p=128))
            nc.sync.dma_start(out=scn_sb, in_=scn.rearrange("(i p) d -> p i d", p=128))

            mask_sb = persist.tile([128, 128], BF16)
            nc.sync.dma_start(out=mask_sb, in_=masks[:])

            ident_f = persist.tile([128, 128], F32)
            make_identity(nc, ident_f)
            ident = persist.tile([128, 128], BF16)
            nc.vector.tensor_copy(ident, ident_f)

            ones_sb = persist.tile([128, 64], F32)
            nc.vector.memset(ones_sb, 1.0)
            ones_r = persist.tile([128, 64], F32R)
            nc.scalar.copy(ones_r, ones_sb)
            ones_bf = persist.tile([128, 1], BF16)
            nc.vector.tensor_copy(ones_bf, ones_sb[:, 0:1])

            eps_sb = persist.tile([128, 1], F32)
            nc.vector.memset(eps_sb, EPS / (QK * QK))

            # V' with ones column: [128, i, h, 65]; fill the ones columns
            # with one strided ACT broadcast copy
            vp_sb = persist.tile([128, NTT, HPC, 65], BF16)
            nc.scalar.copy(
                vp_sb[:, :, :, 64:65],
                ones_bf.unsqueeze(1).broadcast_to([128, NTT, HPC, 1]))

            # Q^T / K^T duplicated across partition halves: [128, {q,k}, h, T]
            qkt_d = persist.tile([128, 2, HPC, T], BF16)
            # Y^T: either per-head [64, h, T] (3stream) or heads 0,1 stacked
            # on partition halves + head 2 separate (stacked)
            if CPROJ == "stacked":
                yt01 = persist.tile([128, T], BF16)
                yt2 = persist.tile([64, T], BF16)
            else:
                yt_d = persist.tile([64, HPC, T], BF16)

            xT_sb = persist.tile([128, NKD, T], BF16)
            xT_r = xT.rearrange("(k p) t -> p k t", p=128)
            for k in range(NKD):
                eng = nc.sync if k % 2 == 0 else nc.gpsimd
                eng.dma_start(out=xT_sb[:, k, :], in_=xT_r[:, k, :])

            normed_mul = (nc.gpsimd.tensor_mul if NORMED_ENG == "gpsimd"
                          else nc.vector.tensor_mul)

            def stage_a(i):
                isl = ts(i, 128)
                ps_qk = proj_ps.tile([128, 384], F32, tag="psqk", name="ps_qk")
                ps_v = tr_ps.tile([128, 192], F32, tag="tr", name="ps_v")
                for k in range(NKD):
                    lhsT = xT_sb[:, k, isl]
                    st = dict(start=(k == 0), stop=(k == NKD - 1))
                    nc.tensor.matmul(ps_qk, lhsT, wqkv_sb[:, k, 0:384], **st)
                    nc.tensor.matmul(ps_v, lhsT, wqkv_sb[:, k, 384:WQW], **st)

                # V -> V': one strided copy into the 65-wide slots
                vp_eng = nc.scalar.copy if "vp" in EVICT_ACT \
                    else nc.vector.tensor_copy
                vp_eng(
                    vp_sb[:, i, :, 0:64],
                    ps_v.rearrange("p (h e) -> p h e", e=64))

                # evict q|k to bf16 SBUF, layout [p, w(q/k), h, two, e]
                qk = misc.tile([128, 2, HPC, 2, 32], BF16, tag="qk", name="qk")
                qk_eng = nc.scalar.copy if "qk" in EVICT_ACT \
                    else nc.vector.tensor_copy
                qk_eng(
                    qk, ps_qk.rearrange("p (w h two e) -> p w h two e",
                                        w=2, h=HPC, two=2))

                # rms sum-of-squares PRE-rope (rotation preserves norms)
                ssum = misc.tile([128, 2, HPC], F32, tag="ssum", name="ssum")
                if SUMSQ == "ttr":
                    sqj = misc.tile([128, 2, HPC, 2, 32], BF16, tag="sqj",
                                    name="sqj")
                    sflat = ssum.rearrange("p w h -> p (w h)")
                    for w in range(2):
                        for h in range(HPC):
                            idx = w * HPC + h
                            nc.vector.tensor_tensor_reduce(
                                out=sqj[:, w, h, :, :],
                                in0=qk[:, w, h, :, :], in1=qk[:, w, h, :, :],
                                scale=1.0, scalar=0.0,
                                op0=mybir.AluOpType.mult,
                                op1=mybir.AluOpType.add,
                                accum_out=sflat[:, idx:idx + 1])
                else:
                    sqj = misc.tile([128, 2, HPC, 64], BF16, tag="sqj",
                                    name="sqj")
                    qkh = qk.rearrange("p w h two e -> p w h (two e)")
                    for w in range(2):
                        # square on GpSimd (plain APs) to offload the DVE
                        nc.gpsimd.tensor_mul(sqj[:, w], qkh[:, w], qkh[:, w])
                        nc.vector.reduce_sum(ssum[:, w], sqj[:, w],
                                             axis=mybir.AxisListType.X)

                # fused q+k rope: 3 DVE ops over [128, 2, 3, 2, 32]
                full = [128, 2, HPC, 2, 32]
                x1 = qk[:, :, :, 0:1, :].broadcast_to(full)
                x2 = qk[:, :, :, 1:2, :].broadcast_to(full)
                cs = csn_sb[:, i, :].rearrange("p (two e) -> p two e", two=2) \
                    .unsqueeze(1).unsqueeze(1).broadcast_to(full)
                sc = scn_sb[:, i, :].rearrange("p (two e) -> p two e", two=2) \
                    .unsqueeze(1).unsqueeze(1).broadcast_to(full)
                rope_eng = nc.gpsimd if ROPE_ENG == "gpsimd" else nc.vector
                t1 = misc.tile(full, BF16, tag="t1", name="t1")
                t2 = misc.tile(full, BF16, tag="t2", name="t2")
                rope_eng.tensor_mul(t1, x1, cs)
                rope_eng.tensor_mul(t2, x2, sc)
                roped = misc.tile([128, 2, HPC, HD], BF16, tag="roped",
                                  name="roped")
                rope_eng.tensor_add(
                    roped.rearrange("p w h (two e) -> p w h two e", two=2),
                    t1, t2)

                # rstd = (mean_sq + eps)^-1/2 computed as exp(-0.5*ln(x)):
                # ln/exp share one ACT table set with the attention exp, so
                # the interleaved schedule never thrashes table loads
                # (sqrt lives in a different set; each switch is ~2.7us).
                # ... entirely on DVE: 1/x via the exact HW reciprocal,
                # sqrt(x) via 2 Babylonian iterations, rstd = (1/x)*sqrt(x).
                # Keeping ScalarE free of non-Exp transcendentals means ONE
                # activation-table load for the whole kernel -- mixing Sqrt
                # with the attention Exp in the interleaved schedule
                # thrashed ~2.7us table loads per op.
                rstd = misc.tile([128, 2, HPC], F32, tag="rstd", name="rstd")
                if RSTD == "lnexp":
                    lnv = misc.tile([128, 2, HPC], F32, tag="lnv", name="lnv")
                    nc.scalar.activation(
                        lnv, ssum, mybir.ActivationFunctionType.Ln,
                        scale=1.0 / (HD * QK * QK), bias=eps_sb)
                    nc.scalar.activation(
                        rstd, lnv, mybir.ActivationFunctionType.Exp,
                        scale=-0.5)
                else:
                    xp = misc.tile([128, 2, HPC], F32, tag="xp", name="xp")
                    nc.vector.tensor_scalar(
                        out=xp, in0=ssum, scalar1=1.0 / (HD * QK * QK),
                        scalar2=EPS / (QK * QK),
                        op0=mybir.AluOpType.mult, op1=mybir.AluOpType.add)
                    zr = misc.tile([128, 2, HPC], F32, tag="zr", name="zr")
                    nc.vector.reciprocal(zr, xp)
                    sq = misc.tile([128, 2, HPC], F32, tag="sqb", name="sqb")
                    nc.vector.tensor_scalar(
                        out=sq, in0=xp, scalar1=0.5, scalar2=0.25,
                        op0=mybir.AluOpType.mult, op1=mybir.AluOpType.add)
                    rr = misc.tile([128, 2, HPC], F32, tag="rr", name="rr")
                    tt = misc.tile([128, 2, HPC], F32, tag="tt", name="tt")
                    for _ in range(2):
                        nc.vector.reciprocal(rr, sq)
                        nc.vector.tensor_mul(tt, xp, rr)
                        nc.vector.tensor_add(sq, sq, tt)
                        nc.vector.tensor_scalar_mul(sq, sq, 0.5)
                    nc.vector.tensor_mul(rstd, zr, sq)

                # normalize, writing the head slice DUPLICATED twice so the
                # transpose below gets a contiguous 128-wide stationary
                normed = misc.tile([128, 2, HPC, 2, HD], BF16, tag="normed",
                                   name="normed")
                normed_mul(
                    normed,
                    roped.unsqueeze(3).broadcast_to([128, 2, HPC, 2, HD]),
                    rstd.unsqueeze(3).unsqueeze(3)
                        .broadcast_to([128, 2, HPC, 2, HD]))

                # PE transposes, duplicated to both partition halves
                ptr = tr_ps.tile([128, 2, HPC, 128], BF16, tag="tr", name="ptr")
                for w in range(2):
                    for h in range(HPC):
                        in_dup = normed[:, w, h, :, :] \
                            .rearrange("p a e -> p (a e)")
                        nc.tensor.transpose(ptr[:, w, h, :], in_dup, ident)
                qkt_eng = nc.scalar.copy if ("qkt" in EVICT_ACT and i < 8) \
                    else nc.vector.tensor_copy
                qkt_eng(qkt_d[:, :, :, isl], ptr)

            def stage_a_group(i0, g):
                """stage_a over tiles i0..i0+g-1 with the small elementwise
                ops (sumsq, reduce, rstd chain) batched across the group:
                identical arithmetic, ~2x fewer DVE instructions."""
                qkg = misc.tile([128, g, 2, HPC, 2, 32], BF16, tag="qk",
                                name="qkg")
                for t in range(g):
                    i = i0 + t
                    isl = ts(i, 128)
                    ps_qk = proj_ps.tile([128, 384], F32, tag="psqk",
                                         name="ps_qk")
                    ps_v = tr_ps.tile([128, 192], F32, tag="tr", name="ps_v")
                    for k in range(NKD):
                        lhsT = xT_sb[:, k, isl]
                        st = dict(start=(k == 0), stop=(k == NKD - 1))
                        nc.tensor.matmul(ps_qk, lhsT, wqkv_sb[:, k, 0:384],
                                         **st)
                        nc.tensor.matmul(ps_v, lhsT, wqkv_sb[:, k, 384:WQW],
                                         **st)
                    vp_eng = nc.scalar.copy if "vp" in EVICT_ACT \
                        else nc.vector.tensor_copy
                    vp_eng(vp_sb[:, i, :, 0:64],
                           ps_v.rearrange("p (h e) -> p h e", e=64))
                    qk_eng = nc.scalar.copy if "qk" in EVICT_ACT \
                        else nc.vector.tensor_copy
                    qk_eng(qkg[:, t],
                           ps_qk.rearrange("p (w h two e) -> p w h two e",
                                           w=2, h=HPC, two=2))

                # batched sum-of-squares PRE-rope (rotation preserves norms)
                sqj = misc.tile([128, g, 2, HPC, HD], BF16, tag="sqj",
                                name="sqj")
                qkh = qkg.rearrange("p g w h two e -> p g w h (two e)")
                nc.gpsimd.tensor_mul(sqj, qkh, qkh)
                ssum = misc.tile([128, g, 2, HPC], F32, tag="ssum",
                                 name="ssum")
                nc.vector.reduce_sum(ssum, sqj, axis=mybir.AxisListType.X)

                # batched rstd chain on [128, g, 2, HPC]
                S = [128, g, 2, HPC]
                rstd = misc.tile(S, F32, tag="rstd", name="rstd")
                xp = misc.tile(S, F32, tag="xp", name="xp")
                nc.vector.tensor_scalar(
                    out=xp, in0=ssum, scalar1=1.0 / (HD * QK * QK),
                    scalar2=EPS / (QK * QK),
                    op0=mybir.AluOpType.mult, op1=mybir.AluOpType.add)
                zr = misc.tile(S, F32, tag="zr", name="zr")
                nc.vector.reciprocal(zr, xp)
                sq = misc.tile(S, F32, tag="sqb", name="sqb")
                nc.vector.tensor_scalar(
                    out=sq, in0=xp, scalar1=0.5, scalar2=0.25,
                    op0=mybir.AluOpType.mult, op1=mybir.AluOpType.add)
                rr = misc.tile(S, F32, tag="rr", name="rr")
                tt = misc.tile(S, F32, tag="tt", name="tt")
                for _ in range(2):
                    nc.vector.reciprocal(rr, sq)
                    nc.vector.tensor_mul(tt, xp, rr)
                    nc.vector.tensor_add(sq, sq, tt)
                    nc.vector.tensor_scalar_mul(sq, sq, 0.5)
                nc.vector.tensor_mul(rstd, zr, sq)

                for t in range(g):
                    i = i0 + t
                    isl = ts(i, 128)
                    qk = qkg[:, t]
                    full = [128, 2, HPC, 2, 32]
                    x1 = qk[:, :, :, 0:1, :].broadcast_to(full)
                    x2 = qk[:, :, :, 1:2, :].broadcast_to(full)
                    cs = csn_sb[:, i, :] \
                        .rearrange("p (two e) -> p two e", two=2) \
                        .unsqueeze(1).unsqueeze(1).broadcast_to(full)
                    sc = scn_sb[:, i, :] \
                        .rearrange("p (two e) -> p two e", two=2) \
                        .unsqueeze(1).unsqueeze(1).broadcast_to(full)
                    t1 = misc.tile(full, BF16, tag="t1", name="t1")
                    t2 = misc.tile(full, BF16, tag="t2", name="t2")
                    nc.vector.tensor_mul(t1, x1, cs)
                    nc.vector.tensor_mul(t2, x2, sc)
                    roped = misc.tile([128, 2, HPC, HD], BF16, tag="roped",
                                      name="roped")
                    nc.vector.tensor_add(
                        roped.rearrange("p w h (two e) -> p w h two e",
                                        two=2),
                        t1, t2)
                    normed = misc.tile([128, 2, HPC, 2, HD], BF16,
                                       tag="normed", name="normed")
                    normed_mul(
                        normed,
                        roped.unsqueeze(3).broadcast_to([128, 2, HPC, 2, HD]),
                        rstd[:, t].unsqueeze(3).unsqueeze(3)
                            .broadcast_to([128, 2, HPC, 2, HD]))
                    ptr = tr_ps.tile([128, 2, HPC, 128], BF16, tag="tr",
                                     name="ptr")
                    for w in range(2):
                        for h in range(HPC):
                            in_dup = normed[:, w, h, :, :] \
                                .rearrange("p a e -> p (a e)")
                            nc.tensor.transpose(ptr[:, w, h, :], in_dup, ident)
                    qkt_eng = nc.scalar.copy \
                        if ("qkt" in EVICT_ACT and i < 8) \
                        else nc.vector.tensor_copy
                    qkt_eng(qkt_d[:, :, :, isl], ptr)

            def attention(q0, w):
                qsl = slice(q0, q0 + w)
                t0 = q0 // 128           # chunk's first q-tile index
                njt = (q0 + w) // 128    # causal: k-tiles overlapping chunk
                for h in range(HPC):
                    qt_h = qkt_d[:, 0, h, :]
                    kt_h = qkt_d[:, 1, h, :]
                    po = opp_ps.tile([65, QCH], F32, tag="opp", name="po")
                    for j0 in range(0, njt, 2):
                        pair = [j for j in (j0, j0 + 1) if j < njt]
                        ps_s = attn_ps.tile([128, 2, QCH], F32, tag="ps_s",
                                            name="ps_s")
                        p_t = p_pool.tile([128, 2, QCH], BF16, tag="pt",
                                          name="p_t")
                        info = []
                        for idx, j in enumerate(pair):
                            s = j - t0
                            c0 = 128 * s if s > 0 else 0
                            info.append((j, idx, s, c0))
                            lo, hi = 64 * idx, 64 * (idx + 1)
                            nc.tensor.matmul(
                                ps_s[:, idx, c0:w],
                                kt_h[lo:hi, ts(j, 128)],
                                qt_h[lo:hi, q0 + c0:q0 + w],
                                start=True, stop=True)
                        # Exp the two-bank pair in one batched ACT op, but
                        # start at the FIRST tile's diagonal offset c0min:
                        # columns below it are never read by either tile's
                        # PV matmul (each slices its own c0:w), so skipping
                        # them trims the exp volume at no extra op count.
                        if len(pair) == 2:
                            c0min = info[0][3] if C0MIN else 0
                            nc.scalar.activation(p_t[:, :, c0min:w],
                                                 ps_s[:, :, c0min:w],
                                                 mybir.ActivationFunctionType.Exp,
                                                 scale=float(HD) ** -0.5)
                        else:
                            for j, idx, s, c0 in info:
                                nc.scalar.activation(
                                    p_t[:, idx, c0:w], ps_s[:, idx, c0:w],
                                    mybir.ActivationFunctionType.Exp,
                                    scale=float(HD) ** -0.5)
                        for j, idx, s, c0 in info:
                            if s >= 0:
                                nc.gpsimd.tensor_mul(p_t[:, idx, c0:c0 + 128],
                                                     p_t[:, idx, c0:c0 + 128],
                                                     mask_sb)
                            nc.tensor.matmul(po[:, c0:w], vp_sb[:, j, h, :],
                                             p_t[:, idx, c0:w],
                                             start=(j == 0), stop=(j == njt - 1))
                    # normalize: 1/l, partition-broadcast via ones matmul,
                    # then one multiply into Y^T
                    recip = misc.tile([65, QCH], F32R, tag="recip", name="recip")
                    with nc.allow_low_precision(reason="f32r softmax denom"):
                        nc.vector.reciprocal(recip[64:65, 0:w], po[64:65, 0:w])
                    pb = opp_ps.tile([64, QCH], F32, tag="opp", name="pb")
                    nc.tensor.matmul(pb[:, 0:w], ones_r[64:65, :],
                                     recip[64:65, 0:w], start=True, stop=True)
                    bcast = misc.tile([64, QCH], BF16, tag="bcast", name="bcast")
                    nc.vector.tensor_copy(bcast[:, 0:w], pb[:, 0:w])
                    if CPROJ != "stacked":
                        nc.vector.tensor_mul(yt_d[:, h, qsl], po[0:64, 0:w],
                                             bcast[:, 0:w])
                    elif h == 0:
                        nc.vector.tensor_mul(yt01[0:64, qsl], po[0:64, 0:w],
                                             bcast[:, 0:w])
                    elif h == 1:
                        yst = misc.tile([64, QCH], BF16, tag="yst", name="yst")
                        nc.vector.tensor_mul(yst[:, 0:w], po[0:64, 0:w],
                                             bcast[:, 0:w])
                        nc.sync.dma_start(out=yt01[64:128, qsl],
                                          in_=yst[:, 0:w])
                    else:
                        nc.vector.tensor_mul(yt2[:, qsl], po[0:64, 0:w],
                                             bcast[:, 0:w])

            def cproj(q0, w):
                qsl = slice(q0, q0 + w)
                for m in range(D // 128):
                    pp = opp_ps.tile([128, QCH], F32, tag="opp", name="pp")
                    if CPROJ == "stacked":
                        nc.tensor.matmul(pp[:, 0:w], wo01_sb[:, ts(m, 128)],
                                         yt01[:, qsl], start=True, stop=False)
                        nc.tensor.matmul(pp[:, 0:w], wo2_sb[:, ts(m, 128)],
                                         yt2[:, qsl], start=False, stop=True)
                    else:
                        for h in range(HPC):
                            nc.tensor.matmul(pp[:, 0:w], wo_d[:, h, ts(m, 128)],
                                             yt_d[:, h, qsl],
                                             start=(h == 0), stop=(h == HPC - 1))
                    ot = co_pool.tile([128, QCH], F32, tag="ot", name="ot")
                    # phase-aware eviction: early chunks run while stage A
                    # keeps DVE pegged (ACT has slack); late chunks sit in
                    # the exp-bound tail where DVE idles.
                    use_act = (q0 < 1024 and m % 2 == 0) if PHEV \
                        else (m % 2 == 0)
                    if use_act:
                        nc.scalar.copy(ot[:, 0:w], pp[:, 0:w])
                    else:
                        nc.vector.tensor_copy(ot[:, 0:w], pp[:, 0:w])
                    nc.sync.dma_start(out=outT[ts(m, 128), qsl], in_=ot[:, 0:w])

            def emit_body():
                cursor = 0
                for q0, w in CHUNKS:
                    need = (q0 + w) // 128
                    if stages in ("all", "a"):
                        if GRP > 0:
                            assert (need - cursor) % GRP == 0
                            for i0 in range(cursor, need, GRP):
                                stage_a_group(i0, GRP)
                        else:
                            for i in range(cursor, need):
                                stage_a(i)
                    cursor = need
                    if stages in ("all", "b"):
                        attention(q0, w)
                        cproj(q0, w)

            if repeat > 1:
                with tc.For_i(0, repeat, 1):
                    emit_body()
            else:
                emit_body()

    nc.finalize()
    return nc


_NC = None


def _get_nc():
    global _NC
    if _NC is None:
        _NC = build_nc()
    return _NC


def _prep_inputs(x, wq, wk, wv, wo, alpha, cos, sin):
    x = np.asarray(x, dtype=np.float32)
    wq = np.asarray(wq, dtype=np.float32)
    wk = np.asarray(wk, dtype=np.float32)
    wv = np.asarray(wv, dtype=np.float32)
    wo = np.asarray(wo, dtype=np.float32)
    alpha = np.asarray(alpha, dtype=np.float32)
    cos = np.asarray(cos, dtype=np.float32)
    sin = np.asarray(sin, dtype=np.float32)

    # softmax over basis heads (fp32, stable)
    a = alpha - alpha.max(axis=-1, keepdims=True)
    e = np.exp(a)
    w = e / e.sum(axis=-1, keepdims=True)          # [H, KH]

    # fold the basis combination into effective per-head wk / wv
    wk_eff = np.einsum("dje,hj->dhe", wk.reshape(D, KH, HD), w).reshape(D, H * HD)
    wv_eff = np.einsum("dje,hj->dhe", wv.reshape(D, KH, HD), w).reshape(D, H * HD)

    csn = np.concatenate([cos, sin], axis=1).astype(NPBF16)      # [T, 64]
    scn = np.concatenate([-sin, cos], axis=1).astype(NPBF16)     # [T, 64]

    # single [128, 128] triangular mask (k <= q) for diagonal sub-blocks
    kk = np.arange(128)[:, None]
    qq = np.arange(128)[None, :]
    masks = np.ascontiguousarray((kk <= qq).astype(NPBF16))

    in_maps = []
    for c in range(NCORES):
        b, g = c // 4, c % 4
        sl = slice(g * HPC * HD, (g + 1) * HPC * HD)
        wqkv = np.zeros((D, WQW), dtype=np.float32)
        wqkv[:, 0:192] = wq[:, sl]
        wqkv[:, 192:384] = wk_eff[:, sl]
        wqkv[:, 384:576] = wv_eff[:, sl]
        wo_c = wo[sl, :]
        wo3 = np.ascontiguousarray(
            wo_c.reshape(HPC, 64, D).transpose(1, 0, 2).reshape(64, HPC * D))
        in_maps.append({
            "xT": np.ascontiguousarray(x[b].T).astype(NPBF16),
            "wqkv": wqkv.astype(NPBF16),
            "wo01": np.ascontiguousarray(wo_c[0:128, :]).astype(NPBF16),
            "wo2": np.ascontiguousarray(wo_c[128:192, :]).astype(NPBF16),
            "wo3": wo3.astype(NPBF16),
            "csn": csn,
            "scn": scn,
            "masks": masks,
        })
    return in_maps


def run(trace=False, **inputs):
    nc = _get_nc()
    in_maps = _prep_inputs(**inputs)
    res = run_bass_kernel_spmd(nc, in_maps, list(range(NCORES)), trace=trace)
    out = np.zeros((B, T, D), dtype=np.float32)
    for c in range(NCORES):
        out[c // 4] += res.results[c]["outT"].T
    return out, res


def kernel(**inputs):
    out, _ = run(**inputs)
    return out



# revision 25
# speedup vs baseline: 1.7626x; 1.7626x over previous
"""Bass/Trainium2 kernel for nn_BQAVariant (BQA: basis-weighted KV attention).

Reference computation (B=2, T=2048, D=768, H=12 q-heads, KH=4 KV basis
heads, HD=64):
  q = x@wq; k_basis = x@wk; v_basis = x@wv
  w = softmax(alpha); k/v = einsum('hj,btjd->bthd', w, {k,v}_basis)
  q,k = rmsnorm(rope(q,k)) * 1.2
  y = causal_sdpa(q, k, v, scale=HD**-0.5) @ wo

Sharding: 24 (batch, head) pairs over 8 cores -> core c handles batch c//4
and heads {3g, 3g+1, 3g+2} with g = c%4.  The alpha-softmax basis
combination is folded on the host into effective per-head wk/wv, so each
core runs three independent standard attention heads.  Each core emits its
partial c_proj output (transposed, [768, 2048]); the host sums the 4
partials per batch.

v2 design (bf16 data, fp32 PSUM accumulation), ~126us/iter HW-measured
(difference timing, R=1 vs R=401 on-device repeat loop):
  A) QKV projections from pre-transposed bf16 xT with packed [768, 576]
     bf16 weights; PSUM evicted once to bf16; RMS sum-of-squares computed
     PRE-rope (rotation preserves norms; cos^2+sin^2=1); fused q+k RoPE
     (3 DVE ops); rsqrt computed ENTIRELY on DVE (reciprocal + 2
     Babylonian sqrt iterations) so ScalarE runs nothing but Exp/Copy ->
     exactly ONE activation-table load for the whole kernel (Sqrt lives in
     a different ACT table set than Exp; in the interleaved schedule each
     sqrt<->exp alternation cost a ~2.7us table reload -- the fp32r
     baseline paid 11 loads, a naive interleave 33).  Q^T/K^T written
     DUPLICATED across both partition halves (normalize-mul writes each
     head twice; one PE transpose per (q/k, head) with a contiguous
     128-wide stationary).
  B) Transposed flash attention: consecutive causal k-tiles (j, j+1) run
     CONCURRENTLY in disjoint PE row-groups (tile_position via base
     partition 0/64 of the duplicated Q^T/K^T halves) -> ~2x S matmul
     throughput.  exp() on ScalarE in [128,2,512] two-bank batches; V'
     carries a ones column so the softmax denominator falls out of the PV
     matmul; the denominator reciprocal is partition-broadcast with a
     K=1 ones matmul.
  C) c_proj: three K=64 matmuls accumulating one PSUM bank; eviction
     alternates ScalarE/VectorE.  Stage A emission is interleaved with
     attention chunks (4 t-tiles, then one 512-wide q-chunk) so
     PE/DVE/ACT/GpSimd overlap across phases.  8 PSUM banks:
     proj 1 + projv 1 + tr/pb 1 + scores 2x2 + po/pp 1.
"""

import os
import sys

sys.path.insert(0, "/opt/trn_rl_repo")

import numpy as np
import ml_dtypes

import concourse.bass as bass
import concourse.tile as tile
from concourse import bacc, mybir

# Steer Bacc.insert_act_table_loads toward the one table set that serves
# BOTH Exp and Ln (natural_log_exp_and_others): hide Exp/Ln from every
# other set so the load-insertion fixpoint cannot pick exp_and_others /
# natural_log and thrash ~2.7us table reloads on each Ln<->Exp
# alternation.  The emitted act_func_set_id still indexes the real
# act_info.json list, so the NEFF loads a genuine table that contains
# both functions.
_orig_gat = bacc.get_activation_tables


def _gat_combined(arch):
    out = {}
    for name, funcs in _orig_gat(arch).items():
        f = set(funcs)
        if name != "natural_log_exp_and_others":
            f.discard(mybir.ActivationFunctionType.Exp)
            f.discard(mybir.ActivationFunctionType.Ln)
        out[name] = f
    return out


bacc.get_activation_tables = _gat_combined
from concourse.bass import ts
from concourse.bass_utils import run_bass_kernel_spmd
from concourse.masks import make_identity

F32 = mybir.dt.float32
F32R = mybir.dt.float32r
BF16 = mybir.dt.bfloat16
NPBF16 = ml_dtypes.bfloat16

B, T, D = 2, 2048, 768
H, KH, HD = 12, 4, 64
HPC = 3            # heads per core
NCORES = 8
EPS = 1e-6
QK = 1.2
NTT = T // 128     # 16 T-tiles
NKD = D // 128     # 6 contraction tiles for projections
QCH = 512          # max q-chunk width in flash stage (PSUM bank limit)
# uneven q-chunk widths: big chunks early (where stage A overlaps), small
# final chunk so the un-overlappable exp tail shrinks; all multiples of 128
_CW = [int(x) for x in
       os.environ.get("BQA_CHUNKS", "512,512,512,512").split(",")]
assert sum(_CW) == T and all(w % 128 == 0 and w <= QCH for w in _CW)
CHUNKS = []
_q0 = 0
for _w in _CW:
    CHUNKS.append((_q0, _w))
    _q0 += _w
WQW = 576          # packed projection width: 192 q | 192 k | 192 v

# analysis knob: "all" | "a" (projections only) | "b" (attention+cproj only)
STAGES = os.environ.get("BQA_STAGES", "all")
# hardware-timing knob: repeat the whole body N times inside a For_i loop
REPEAT = int(os.environ.get("BQA_REPEAT", "1"))
# dup-transpose strategy: "bcast" = one PE transpose with stride-0 doubled
# stationary; "two" = two transposes (lower + upper psum half)
DUPT = os.environ.get("BQA_DUPT", "bcast")
# engine for the rmsnorm multiply: "vector" | "gpsimd"
NORMED_ENG = os.environ.get("BQA_NORMED", "vector")
# sum-of-squares: "ttr" (fused tensor_tensor_reduce) | "sq" (mul + reduce)
SUMSQ = os.environ.get("BQA_SUMSQ", "sq")
# cproj contraction: "3stream" (three K=64 matmuls) | "stacked" (K=128+64,
# head 1 moved to upper partitions by SBUF->SBUF DMA)
CPROJ = os.environ.get("BQA_CPROJ", "3stream")
# rstd path: "lnexp" (2 ScalarE ops via exp(-0.5*ln(x)); Ln+Exp share the
# natural_log_exp_and_others table set with the attention Exp, so no table
# thrash) | "dve" (reciprocal + 2 Babylonian sqrt iterations, ~12 DVE ops)
RSTD = os.environ.get("BQA_RSTD", "dve")
# engine for the rope multiplies/add: "vector" | "gpsimd"
ROPE_ENG = os.environ.get("BQA_ROPE", "vector")
# engines for stage-A PSUM evictions: comma list of which go to ScalarE
# (subset of "qk,vp,qkt"); the rest stay on DVE
EVICT_ACT = set(filter(None, os.environ.get("BQA_EVICT_ACT", "").split(",")))
# stage-A group size: 0 = legacy per-tile path; 1/2/4 = batch the sumsq /
# reduce / rstd-chain ops over groups of tiles (identical arithmetic, far
# fewer small DVE instructions)
GRP = int(os.environ.get("BQA_GRP", "0"))
# exp slicing from the pair's first diagonal offset (1) vs full pair (0)
C0MIN = int(os.environ.get("BQA_C0MIN", "0"))
# cproj eviction: 1 = phase-aware (ACT early chunks, DVE late), 0 = m%2
PHEV = int(os.environ.get("BQA_PHEV", "0"))


def build_nc(repeat=None, stages=None):
    repeat = REPEAT if repeat is None else repeat
    stages = STAGES if stages is None else stages
    nc = bacc.Bacc(None, target_bir_lowering=False)

    xT = nc.declare_dram_parameter("xT", [D, T], BF16, isOutput=False)
    wqkv = nc.declare_dram_parameter("wqkv", [D, WQW], BF16, isOutput=False)
    wo01 = nc.declare_dram_parameter("wo01", [128, D], BF16, isOutput=False)
    wo2 = nc.declare_dram_parameter("wo2", [64, D], BF16, isOutput=False)
    wo3 = nc.declare_dram_parameter("wo3", [64, HPC * D], BF16, isOutput=False)
    csn = nc.declare_dram_parameter("csn", [T, HD], BF16, isOutput=False)
    scn = nc.declare_dram_parameter("scn", [T, HD], BF16, isOutput=False)
    masks = nc.declare_dram_parameter("masks", [128, 128], BF16, isOutput=False)
    outT = nc.declare_dram_parameter("outT", [D, T], F32, isOutput=True)

    with tile.TileContext(nc) as tc:
        with (
            tc.tile_pool(name="persist", bufs=1) as persist,
            tc.tile_pool(name="misc", bufs=2) as misc,
            tc.tile_pool(name="p_sb", bufs=3) as p_pool,
            tc.tile_pool(name="co_sb", bufs=2) as co_pool,
            # PSUM: 8 banks: psqk 1 + [psv|tr] 1 + ps_s 2x2 + [po|pb|pp] 2x1
            tc.tile_pool(name="proj_ps", bufs=1, space="PSUM") as proj_ps,
            tc.tile_pool(name="tr_ps", bufs=1, space="PSUM") as tr_ps,
            tc.tile_pool(name="attn_ps", bufs=2, space="PSUM") as attn_ps,
            tc.tile_pool(name="opp_ps", bufs=2, space="PSUM") as opp_ps,
        ):
            # --- persistent SBUF tensors ---
            wqkv_sb = persist.tile([128, NKD, WQW], BF16)
            wqkv_r = wqkv.rearrange("(k p) n -> p k n", p=128)
            for k in range(NKD):
                eng = nc.sync if k % 2 == 0 else nc.gpsimd
                eng.dma_start(out=wqkv_sb[:, k, :], in_=wqkv_r[:, k, :])

            if CPROJ == "stacked":
                wo01_sb = persist.tile([128, D], BF16)
                wo2_sb = persist.tile([64, D], BF16)
                nc.sync.dma_start(out=wo01_sb, in_=wo01[:])
                nc.sync.dma_start(out=wo2_sb, in_=wo2[:])
            else:
                wo_d = persist.tile([64, HPC, D], BF16)
                nc.sync.dma_start(
                    out=wo_d, in_=wo3.rearrange("p (h d) -> p h d", h=HPC))

            csn_sb = persist.tile([128, NTT, HD], BF16)
            scn_sb = persist.tile([128, NTT, HD], BF16)
            nc.sync.dma_start(out=csn_sb, in_=csn.rearrange("(i p) d -> p i d", p=128))
            nc.sync.dma_start(out=scn_sb, in_=scn.rearrange("(i p) d -> p i d", p=128))

            mask_sb = persist.tile([128, 128], BF16)
            nc.sync.dma_start(out=mask_sb, in_=masks[:])

            ident_f = persist.tile([128, 128], F32)
            make_identity(nc, ident_f)
            ident = persist.tile([128, 128], BF16)
            nc.vector.tensor_copy(ident, ident_f)

            ones_sb = persist.tile([128, 64], F32)
            nc.vector.memset(ones_sb, 1.0)
            ones_r = persist.tile([128, 64], F32R)
            nc.scalar.copy(ones_r, ones_sb)
            ones_bf = persist.tile([128, 1], BF16)
            nc.vector.tensor_copy(ones_bf, ones_sb[:, 0:1])

            eps_sb = persist.tile([128, 1], F32)
            nc.vector.memset(eps_sb, EPS / (QK * QK))

            # V' with ones column: [128, i, h, 65]; fill the ones columns
            # with one strided ACT broadcast copy
            vp_sb = persist.tile([128, NTT, HPC, 65], BF16)
            nc.scalar.copy(
                vp_sb[:, :, :, 64:65],
                ones_bf.unsqueeze(1).broadcast_to([128, NTT, HPC, 1]))

            # Q^T / K^T duplicated across partition halves: [128, {q,k}, h, T]
            qkt_d = persist.tile([128, 2, HPC, T], BF16)
            # Y^T: either per-head [64, h, T] (3stream) or heads 0,1 stacked
            # on partition halves + head 2 separate (stacked)
            if CPROJ == "stacked":
                yt01 = persist.tile([128, T], BF16)
                yt2 = persist.tile([64, T], BF16)
            else:
                yt_d = persist.tile([64, HPC, T], BF16)

            xT_sb = persist.tile([128, NKD, T], BF16)
            xT_r = xT.rearrange("(k p) t -> p k t", p=128)
            for k in range(NKD):
                eng = nc.sync if k % 2 == 0 else nc.gpsimd
                eng.dma_start(out=xT_sb[:, k, :], in_=xT_r[:, k, :])

            normed_mul = (nc.gpsimd.tensor_mul if NORMED_ENG == "gpsimd"
                          else nc.vector.tensor_mul)

            def stage_a(i):
                isl = ts(i, 128)
                ps_qk = proj_ps.tile([128, 384], F32, tag="psqk", name="ps_qk")
                ps_v = tr_ps.tile([128, 192], F32, tag="tr", name="ps_v")
                for k in range(NKD):
                    lhsT = xT_sb[:, k, isl]
                    st = dict(start=(k == 0), stop=(k == NKD - 1))
                    nc.tensor.matmul(ps_qk, lhsT, wqkv_sb[:, k, 0:384], **st)
                    nc.tensor.matmul(ps_v, lhsT, wqkv_sb[:, k, 384:WQW], **st)

                # V -> V': one strided copy into the 65-wide slots
                vp_eng = nc.scalar.copy if "vp" in EVICT_ACT \
                    else nc.vector.tensor_copy
                vp_eng(
                    vp_sb[:, i, :, 0:64],
                    ps_v.rearrange("p (h e) -> p h e", e=64))

                # evict q|k to bf16 SBUF, layout [p, w(q/k), h, two, e]
                qk = misc.tile([128, 2, HPC, 2, 32], BF16, tag="qk", name="qk")
                qk_eng = nc.scalar.copy if "qk" in EVICT_ACT \
                    else nc.vector.tensor_copy
                qk_eng(
                    qk, ps_qk.rearrange("p (w h two e) -> p w h two e",
                                        w=2, h=HPC, two=2))

                # rms sum-of-squares PRE-rope (rotation preserves norms)
                ssum = misc.tile([128, 2, HPC], F32, tag="ssum", name="ssum")
                if SUMSQ == "ttr":
                    sqj = misc.tile([128, 2, HPC, 2, 32], BF16, tag="sqj",
                                    name="sqj")
                    sflat = ssum.rearrange("p w h -> p (w h)")
                    for w in range(2):
                        for h in range(HPC):
                            idx = w * HPC + h
                            nc.vector.tensor_tensor_reduce(
                                out=sqj[:, w, h, :, :],
                                in0=qk[:, w, h, :, :], in1=qk[:, w, h, :, :],
                                scale=1.0, scalar=0.0,
                                op0=mybir.AluOpType.mult,
                                op1=mybir.AluOpType.add,
                                accum_out=sflat[:, idx:idx + 1])
                else:
                    sqj = misc.tile([128, 2, HPC, 64], BF16, tag="sqj",
                                    name="sqj")
                    qkh = qk.rearrange("p w h two e -> p w h (two e)")
                    for w in range(2):
                        # square on GpSimd (plain APs) to offload the DVE
                        nc.gpsimd.tensor_mul(sqj[:, w], qkh[:, w], qkh[:, w])
                        nc.vector.reduce_sum(ssum[:, w], sqj[:, w],
                                             axis=mybir.AxisListType.X)

                # fused q+k rope: 3 DVE ops over [128, 2, 3, 2, 32]
                full = [128, 2, HPC, 2, 32]
                x1 = qk[:, :, :, 0:1, :].broadcast_to(full)
                x2 = qk[:, :, :, 1:2, :].broadcast_to(full)
                cs = csn_sb[:, i, :].rearrange("p (two e) -> p two e", two=2) \
                    .unsqueeze(1).unsqueeze(1).broadcast_to(full)
                sc = scn_sb[:, i, :].rearrange("p (two e) -> p two e", two=2) \
                    .unsqueeze(1).unsqueeze(1).broadcast_to(full)
                rope_eng = nc.gpsimd if ROPE_ENG == "gpsimd" else nc.vector
                t1 = misc.tile(full, BF16, tag="t1", name="t1")
                t2 = misc.tile(full, BF16, tag="t2", name="t2")
                rope_eng.tensor_mul(t1, x1, cs)
                rope_eng.tensor_mul(t2, x2, sc)
                roped = misc.tile([128, 2, HPC, HD], BF16, tag="roped",
                                  name="roped")
                rope_eng.tensor_add(
                    roped.rearrange("p w h (two e) -> p w h two e", two=2),
                    t1, t2)

                # rstd = (mean_sq + eps)^-1/2, entirely on DVE: 1/x via the
                # exact HW reciprocal, sqrt(x) via 2 Babylonian iterations,
                # rstd = (1/x)*sqrt(x).  Keeping ScalarE free of non-Exp
                # transcendentals means ONE activation-table load for the
                # whole kernel.  (HW-measured: the exp(-0.5*ln(x)) ScalarE
                # variant [RSTD=lnexp] is +180us -- walrus inserts its own
                # table loads, thrashing ~2.7us on every Ln<->Exp
                # alternation regardless of Bacc's pre-placed loads.)
                rstd = misc.tile([128, 2, HPC], F32, tag="rstd", name="rstd")
                if RSTD == "lnexp":
                    lnv = misc.tile([128, 2, HPC], F32, tag="lnv", name="lnv")
                    nc.scalar.activation(
                        lnv, ssum, mybir.ActivationFunctionType.Ln,
                        scale=1.0 / (HD * QK * QK), bias=eps_sb)
                    nc.scalar.activation(
                        rstd, lnv, mybir.ActivationFunctionType.Exp,
                        scale=-0.5)
                else:
                    xp = misc.tile([128, 2, HPC], F32, tag="xp", name="xp")
                    nc.vector.tensor_scalar(
                        out=xp, in0=ssum, scalar1=1.0 / (HD * QK * QK),
                        scalar2=EPS / (QK * QK),
                        op0=mybir.AluOpType.mult, op1=mybir.AluOpType.add)
                    zr = misc.tile([128, 2, HPC], F32, tag="zr", name="zr")
                    nc.vector.reciprocal(zr, xp)
                    sq = misc.tile([128, 2, HPC], F32, tag="sqb", name="sqb")
                    nc.vector.tensor_scalar(
                        out=sq, in0=xp, scalar1=0.5, scalar2=0.25,
                        op0=mybir.AluOpType.mult, op1=mybir.AluOpType.add)
                    rr = misc.tile([128, 2, HPC], F32, tag="rr", name="rr")
                    tt = misc.tile([128, 2, HPC], F32, tag="tt", name="tt")
                    for _ in range(2):
                        nc.vector.reciprocal(rr, sq)
                        nc.vector.tensor_mul(tt, xp, rr)
                        nc.vector.tensor_add(sq, sq, tt)
                        nc.vector.tensor_scalar_mul(sq, sq, 0.5)
                    nc.vector.tensor_mul(rstd, zr, sq)

                # normalize, writing the head slice DUPLICATED twice so the
                # transpose below gets a contiguous 128-wide stationary
                normed = misc.tile([128, 2, HPC, 2, HD], BF16, tag="normed",
                                   name="normed")
                normed_mul(
                    normed,
                    roped.unsqueeze(3).broadcast_to([128, 2, HPC, 2, HD]),
                    rstd.unsqueeze(3).unsqueeze(3)
                        .broadcast_to([128, 2, HPC, 2, HD]))

                # PE transposes, duplicated to both partition halves
                ptr = tr_ps.tile([128, 2, HPC, 128], BF16, tag="tr", name="ptr")
                for w in range(2):
                    for h in range(HPC):
                        in_dup = normed[:, w, h, :, :] \
                            .rearrange("p a e -> p (a e)")
                        nc.tensor.transpose(ptr[:, w, h, :], in_dup, ident)
                qkt_eng = nc.scalar.copy if ("qkt" in EVICT_ACT and i < 8) \
                    else nc.vector.tensor_copy
                qkt_eng(qkt_d[:, :, :, isl], ptr)

            def stage_a_group(i0, g):
                """stage_a over tiles i0..i0+g-1 with the small elementwise
                ops (sumsq, reduce, rstd chain) batched across the group:
                identical arithmetic, ~2x fewer DVE instructions."""
                qkg = misc.tile([128, g, 2, HPC, 2, 32], BF16, tag="qk",
                                name="qkg")
                for t in range(g):
                    i = i0 + t
                    isl = ts(i, 128)
                    ps_qk = proj_ps.tile([128, 384], F32, tag="psqk",
                                         name="ps_qk")
                    ps_v = tr_ps.tile([128, 192], F32, tag="tr", name="ps_v")
                    for k in range(NKD):
                        lhsT = xT_sb[:, k, isl]
                        st = dict(start=(k == 0), stop=(k == NKD - 1))
                        nc.tensor.matmul(ps_qk, lhsT, wqkv_sb[:, k, 0:384],
                                         **st)
                        nc.tensor.matmul(ps_v, lhsT, wqkv_sb[:, k, 384:WQW],
                                         **st)
                    vp_eng = nc.scalar.copy if "vp" in EVICT_ACT \
                        else nc.vector.tensor_copy
                    vp_eng(vp_sb[:, i, :, 0:64],
                           ps_v.rearrange("p (h e) -> p h e", e=64))
                    qk_eng = nc.scalar.copy if "qk" in EVICT_ACT \
                        else nc.vector.tensor_copy
                    qk_eng(qkg[:, t],
                           ps_qk.rearrange("p (w h two e) -> p w h two e",
                                           w=2, h=HPC, two=2))

                # batched sum-of-squares PRE-rope (rotation preserves norms)
                sqj = misc.tile([128, g, 2, HPC, HD], BF16, tag="sqj",
                                name="sqj")
                qkh = qkg.rearrange("p g w h two e -> p g w h (two e)")
                nc.gpsimd.tensor_mul(sqj, qkh, qkh)
                ssum = misc.tile([128, g, 2, HPC], F32, tag="ssum",
                                 name="ssum")
                nc.vector.reduce_sum(ssum, sqj, axis=mybir.AxisListType.X)

                # batched rstd chain on [128, g, 2, HPC]
                S = [128, g, 2, HPC]
                rstd = misc.tile(S, F32, tag="rstd", name="rstd")
                xp = misc.tile(S, F32, tag="xp", name="xp")
                nc.vector.tensor_scalar(
                    out=xp, in0=ssum, scalar1=1.0 / (HD * QK * QK),
                    scalar2=EPS / (QK * QK),
                    op0=mybir.AluOpType.mult, op1=mybir.AluOpType.add)
                zr = misc.tile(S, F32, tag="zr", name="zr")
                nc.vector.reciprocal(zr, xp)
                sq = misc.tile(S, F32, tag="sqb", name="sqb")
                nc.vector.tensor_scalar(
                    out=sq, in0=xp, scalar1=0.5, scalar2=0.25,
                    op0=mybir.AluOpType.mult, op1=mybir.AluOpType.add)
                rr = misc.tile(S, F32, tag="rr", name="rr")
                tt = misc.tile(S, F32, tag="tt", name="tt")
                for _ in range(2):
                    nc.vector.reciprocal(rr, sq)
                    nc.vector.tensor_mul(tt, xp, rr)
                    nc.vector.tensor_add(sq, sq, tt)
                    nc.vector.tensor_scalar_mul(sq, sq, 0.5)
                nc.vector.tensor_mul(rstd, zr, sq)

                for t in range(g):
                    i = i0 + t
                    isl = ts(i, 128)
                    qk = qkg[:, t]
                    full = [128, 2, HPC, 2, 32]
                    x1 = qk[:, :, :, 0:1, :].broadcast_to(full)
                    x2 = qk[:, :, :, 1:2, :].broadcast_to(full)
                    cs = csn_sb[:, i, :] \
                        .rearrange("p (two e) -> p two e", two=2) \
                        .unsqueeze(1).unsqueeze(1).broadcast_to(full)
                    sc = scn_sb[:, i, :] \
                        .rearrange("p (two e) -> p two e", two=2) \
                        .unsqueeze(1).unsqueeze(1).broadcast_to(full)
                    t1 = misc.tile(full, BF16, tag="t1", name="t1")
                    t2 = misc.tile(full, BF16, tag="t2", name="t2")
                    nc.vector.tensor_mul(t1, x1, cs)
                    nc.vector.tensor_mul(t2, x2, sc)
                    roped = misc.tile([128, 2, HPC, HD], BF16, tag="roped",
                                      name="roped")
                    nc.vector.tensor_add(
                        roped.rearrange("p w h (two e) -> p w h two e",
                                        two=2),
                        t1, t2)
                    normed = misc.tile([128, 2, HPC, 2, HD], BF16,
                                       tag="normed", name="normed")
                    normed_mul(
                        normed,
                        roped.unsqueeze(3).broadcast_to([128, 2, HPC, 2, HD]),
                        rstd[:, t].unsqueeze(3).unsqueeze(3)
                            .broadcast_to([128, 2, HPC, 2, HD]))
                    ptr = tr_ps.tile([128, 2, HPC, 128], BF16, tag="tr",
                                     name="ptr")
                    for w in range(2):
                        for h in range(HPC):
                            in_dup = normed[:, w, h, :, :] \
                                .rearrange("p a e -> p (a e)")
                            nc.tensor.transpose(ptr[:, w, h, :], in_dup, ident)
                    qkt_eng = nc.scalar.copy \
                        if ("qkt" in EVICT_ACT and i < 8) \
                        else nc.vector.tensor_copy
                    qkt_eng(qkt_d[:, :, :, isl], ptr)

            def attention(q0, w):
                qsl = slice(q0, q0 + w)
                t0 = q0 // 128           # chunk's first q-tile index
                njt = (q0 + w) // 128    # causal: k-tiles overlapping chunk
                for h in range(HPC):
                    qt_h = qkt_d[:, 0, h, :]
                    kt_h = qkt_d[:, 1, h, :]
                    po = opp_ps.tile([65, QCH], F32, tag="opp", name="po")
                    for j0 in range(0, njt, 2):
                        pair = [j for j in (j0, j0 + 1) if j < njt]
                        ps_s = attn_ps.tile([128, 2, QCH], F32, tag="ps_s",
                                            name="ps_s")
                        p_t = p_pool.tile([128, 2, QCH], BF16, tag="pt",
                                          name="p_t")
                        info = []
                        for idx, j in enumerate(pair):
                            s = j - t0
                            c0 = 128 * s if s > 0 else 0
                            info.append((j, idx, s, c0))
                            lo, hi = 64 * idx, 64 * (idx + 1)
                            nc.tensor.matmul(
                                ps_s[:, idx, c0:w],
                                kt_h[lo:hi, ts(j, 128)],
                                qt_h[lo:hi, q0 + c0:q0 + w],
                                start=True, stop=True)
                        # Exp the two-bank pair in one batched ACT op, but
                        # start at the FIRST tile's diagonal offset c0min:
                        # columns below it are never read by either tile's
                        # PV matmul (each slices its own c0:w), so skipping
                        # them trims the exp volume at no extra op count.
                        if len(pair) == 2:
                            c0min = info[0][3] if C0MIN else 0
                            nc.scalar.activation(p_t[:, :, c0min:w],
                                                 ps_s[:, :, c0min:w],
                                                 mybir.ActivationFunctionType.Exp,
                                                 scale=float(HD) ** -0.5)
                        else:
                            for j, idx, s, c0 in info:
                                nc.scalar.activation(
                                    p_t[:, idx, c0:w], ps_s[:, idx, c0:w],
                                    mybir.ActivationFunctionType.Exp,
                                    scale=float(HD) ** -0.5)
                        for j, idx, s, c0 in info:
                            if s >= 0:
                                nc.gpsimd.tensor_mul(p_t[:, idx, c0:c0 + 128],
                                                     p_t[:, idx, c0:c0 + 128],
                                                     mask_sb)
                            nc.tensor.matmul(po[:, c0:w], vp_sb[:, j, h, :],
                                             p_t[:, idx, c0:w],
                                             start=(j == 0), stop=(j == njt - 1))
                    # normalize: 1/l, partition-broadcast via ones matmul,
                    # then one multiply into Y^T
                    recip = misc.tile([65, QCH], F32R, tag="recip", name="recip")
                    with nc.allow_low_precision(reason="f32r softmax denom"):
                        nc.vector.reciprocal(recip[64:65, 0:w], po[64:65, 0:w])
                    pb = opp_ps.tile([64, QCH], F32, tag="opp", name="pb")
                    nc.tensor.matmul(pb[:, 0:w], ones_r[64:65, :],
                                     recip[64:65, 0:w], start=True, stop=True)
                    bcast = misc.tile([64, QCH], BF16, tag="bcast", name="bcast")
                    nc.vector.tensor_copy(bcast[:, 0:w], pb[:, 0:w])
                    if CPROJ != "stacked":
                        nc.vector.tensor_mul(yt_d[:, h, qsl], po[0:64, 0:w],
                                             bcast[:, 0:w])
                    elif h == 0:
                        nc.vector.tensor_mul(yt01[0:64, qsl], po[0:64, 0:w],
                                             bcast[:, 0:w])
                    elif h == 1:
                        yst = misc.tile([64, QCH], BF16, tag="yst", name="yst")
                        nc.vector.tensor_mul(yst[:, 0:w], po[0:64, 0:w],
                                             bcast[:, 0:w])
                        nc.sync.dma_start(out=yt01[64:128, qsl],
                                          in_=yst[:, 0:w])
                    else:
                        nc.vector.tensor_mul(yt2[:, qsl], po[0:64, 0:w],
                                             bcast[:, 0:w])

            def cproj(q0, w):
                qsl = slice(q0, q0 + w)
                for m in range(D // 128):
                    pp = opp_ps.tile([128, QCH], F32, tag="opp", name="pp")
                    if CPROJ == "stacked":
                        nc.tensor.matmul(pp[:, 0:w], wo01_sb[:, ts(m, 128)],
                                         yt01[:, qsl], start=True, stop=False)
                        nc.tensor.matmul(pp[:, 0:w], wo2_sb[:, ts(m, 128)],
                                         yt2[:, qsl], start=False, stop=True)
                    else:
                        for h in range(HPC):
                            nc.tensor.matmul(pp[:, 0:w], wo_d[:, h, ts(m, 128)],
                                             yt_d[:, h, qsl],
                                             start=(h == 0), stop=(h == HPC - 1))
                    ot = co_pool.tile([128, QCH], F32, tag="ot", name="ot")
                    # phase-aware eviction: early chunks run while stage A
                    # keeps DVE pegged (ACT has slack); late chunks sit in
                    # the exp-bound tail where DVE idles.
                    use_act = (q0 < 1024 and m % 2 == 0) if PHEV \
                        else (m % 2 == 0)
                    if use_act:
                        nc.scalar.copy(ot[:, 0:w], pp[:, 0:w])
                    else:
                        nc.vector.tensor_copy(ot[:, 0:w], pp[:, 0:w])
                    nc.sync.dma_start(out=outT[ts(m, 128), qsl], in_=ot[:, 0:w])

            def emit_body():
                cursor = 0
                for q0, w in CHUNKS:
                    need = (q0 + w) // 128
                    if stages in ("all", "a"):
                        if GRP > 0:
                            assert (need - cursor) % GRP == 0
                            for i0 in range(cursor, need, GRP):
                                stage_a_group(i0, GRP)
                        else:
                            for i in range(cursor, need):
                                stage_a(i)
                    cursor = need
                    if stages in ("all", "b"):
                        attention(q0, w)
                        cproj(q0, w)

            if repeat > 1:
                with tc.For_i(0, repeat, 1):
                    emit_body()
            else:
                emit_body()

    nc.finalize()
    return nc


_NC = None


def _get_nc():
    global _NC
    if _NC is None:
        _NC = build_nc()
    return _NC


def _prep_inputs(x, wq, wk, wv, wo, alpha, cos, sin):
    x = np.asarray(x, dtype=np.float32)
    wq = np.asarray(wq, dtype=np.float32)
    wk = np.asarray(wk, dtype=np.float32)
    wv = np.asarray(wv, dtype=np.float32)
    wo = np.asarray(wo, dtype=np.float32)
    alpha = np.asarray(alpha, dtype=np.float32)
    cos = np.asarray(cos, dtype=np.float32)
    sin = np.asarray(sin, dtype=np.float32)

    # softmax over basis heads (fp32, stable)
    a = alpha - alpha.max(axis=-1, keepdims=True)
    e = np.exp(a)
    w = e / e.sum(axis=-1, keepdims=True)          # [H, KH]

    # fold the basis combination into effective per-head wk / wv
    wk_eff = np.einsum("dje,hj->dhe", wk.reshape(D, KH, HD), w).reshape(D, H * HD)
    wv_eff = np.einsum("dje,hj->dhe", wv.reshape(D, KH, HD), w).reshape(D, H * HD)

    csn = np.concatenate([cos, sin], axis=1).astype(NPBF16)      # [T, 64]
    scn = np.concatenate([-sin, cos], axis=1).astype(NPBF16)     # [T, 64]

    # single [128, 128] triangular mask (k <= q) for diagonal sub-blocks
    kk = np.arange(128)[:, None]
    qq = np.arange(128)[None, :]
    masks = np.ascontiguousarray((kk <= qq).astype(NPBF16))

    in_maps = []
    for c in range(NCORES):
        b, g = c // 4, c % 4
        sl = slice(g * HPC * HD, (g + 1) * HPC * HD)
        wqkv = np.zeros((D, WQW), dtype=np.float32)
        wqkv[:, 0:192] = wq[:, sl]
        wqkv[:, 192:384] = wk_eff[:, sl]
        wqkv[:, 384:576] = wv_eff[:, sl]
        wo_c = wo[sl, :]
        wo3 = np.ascontiguousarray(
            wo_c.reshape(HPC, 64, D).transpose(1, 0, 2).reshape(64, HPC * D))
        in_maps.append({
            "xT": np.ascontiguousarray(x[b].T).astype(NPBF16),
            "wqkv": wqkv.astype(NPBF16),
            "wo01": np.ascontiguousarray(wo_c[0:128, :]).astype(NPBF16),
            "wo2": np.ascontiguousarray(wo_c[128:192, :]).astype(NPBF16),
            "wo3": wo3.astype(NPBF16),
            "csn": csn,
            "scn": scn,
            "masks": masks,
        })
    return in_maps


def run(trace=False, **inputs):
    nc = _get_nc()
    in_maps = _prep_inputs(**inputs)
    res = run_bass_kernel_spmd(nc, in_maps, list(range(NCORES)), trace=trace)
    out = np.zeros((B, T, D), dtype=np.float32)
    for c in range(NCORES):
        out[c // 4] += res.results[c]["outT"].T
    return out, res


def kernel(**inputs):
    out, _ = run(**inputs)
    return out

